# revision 32
# baseline (speedup 1.0000x reference)
"""CGCNN (3x CGConv + MLP head) Trainium2 kernel, 8-core edge-parallel.

Strategy:
  - Edges sorted by dst; node space padded to 128*SPC*8 and split into
    128-node "blocks" (stripes). Core k owns SPC consecutive blocks.
  - Node features live in SBUF as a bf16 "node-packed" table
    (node n -> partition n%128, free elems [(n//128)*128, +128)); per-edge
    features are fetched with gpsimd.dma_gather (SBUF source, transposed)
    giving feature-major [128, ncols] tiles directly.
  - Messages: per 128-edge subtile, 3 accumulating matmuls
    (dst-part, src-part, edge-attr-part; K=128/128/48) -> PSUM [128e, 184]
    = [lin_f | lin_s]; sigmoid/softplus on ScalarE; product on VectorE.
  - Aggregation: per block, one-hot(is_equal vs iota) matmul accumulated
    over the block's subtiles into PSUM [128 nodes, 92]; residual + SiLU,
    fp32 state kept per-core, bf16 copy into the gather tables.
  - Between convs the bf16 slice tables are AllGathered (SBUF->SBUF
    collective) into the full replicated table.
  - Head: fc1 twice on each core's node slice (feature-major, fp32),
    masked mean-pool partial, AllReduce, then the small MLP replicated.
"""

import numpy as np
import ml_dtypes

import concourse.bass as bass
import concourse.bacc as bacc
import concourse.mybir as mybir
import concourse.tile as tile
from concourse.bass_utils import run_bass_kernel_spmd

BF16 = mybir.dt.bfloat16
F32 = mybir.dt.float32
I16 = mybir.dt.int16
AF = mybir.ActivationFunctionType
ALU = mybir.AluOpType

NCORES = 8
P = 128
NF = 92
EF = 41
EFP = 48  # padded edge-attr rows (41 real + pad, row 47 = ones for bias)


def _rup(x, m):
    return (x + m - 1) // m * m


def _bf16(a):
    return np.asarray(a, dtype=ml_dtypes.bfloat16)


def preprocess(x, edge_index, edge_attr, n_nodes, n_edges):
    """Host-side: sort/pad edges, build all per-core input arrays."""
    spc = _rup(n_nodes, P * NCORES) // (P * NCORES)  # stripes per core
    stripes = spc * NCORES
    npad = stripes * P
    npc = spc * P  # nodes per core
    split = (stripes // 2) * P  # src table split point
    assert split <= 32767 and npad - split <= 32767

    src = np.asarray(edge_index[0], dtype=np.int64)
    dst = np.asarray(edge_index[1], dtype=np.int64)
    ea = np.asarray(edge_attr, dtype=np.float32)

    order = np.argsort(dst, kind="stable")
    src = src[order]
    dst = dst[order]
    ea = ea[order]

    bounds = np.searchsorted(dst, np.arange(0, npad + 1, P))

    # per-(core, block): sort by src, split lo/hi
    blk = []  # (lo_src, lo_dst, hi_src, hi_dst, lo_ea, hi_ea)
    max_lo = max_hi = 0
    for b in range(stripes):
        e0, e1 = bounds[b], bounds[b + 1]
        s, d, a = src[e0:e1], dst[e0:e1], ea[e0:e1]
        o = np.argsort(s, kind="stable")
        s, d, a = s[o], d[o], a[o]
        nlo = int(np.searchsorted(s, split))
        blk.append((s[:nlo], d[:nlo], s[nlo:], d[nlo:], a[:nlo], a[nlo:]))
        max_lo = max(max_lo, nlo)
        max_hi = max(max_hi, len(s) - nlo)

    HLO = max(_rup(max_lo, P), P)
    HHI = max(_rup(max_hi, P), P)
    C = HLO + HHI
    if (C // P) % 2:
        HHI += P
        C += P
    S = C // P  # subtiles per block

    # per-core arrays
    cores = []
    for c in range(NCORES):
        srcidx = np.zeros((stripes // NCORES, C), dtype=np.int16)
        dstidx = np.zeros((stripes // NCORES, C), dtype=np.int16)
        dstmod = np.full((stripes // NCORES, C), -1.0, dtype=np.float32)
        eat = np.zeros((spc, EFP, C), dtype=np.float32)
        for bb in range(spc):
            b = c * spc + bb
            base = b * P
            ls, ld, hs, hd, la, ha = blk[b]
            nlo, nhi = len(ls), len(hs)
            srcidx[bb, :nlo] = ls
            srcidx[bb, HLO : HLO + nhi] = hs - split
            dstidx[bb, :nlo] = ld - c * npc
            dstidx[bb, HLO : HLO + nhi] = hd - c * npc
            dstmod[bb, :nlo] = ld - base
            dstmod[bb, HLO : HLO + nhi] = hd - base
            eat[bb, :EF, :nlo] = la.T
            eat[bb, :EF, HLO : HLO + nhi] = ha.T
            eat[bb, EFP - 1, :nlo] = 1.0
            eat[bb, EFP - 1, HLO : HLO + nhi] = 1.0

        # wrapped-16 idx layout, replicated to 128 partitions
        def wrap16(arr):  # [nblk, C] -> [128, nblk*C//16]
            w = arr.reshape(-1, 16).T  # [16, nblk*C/16]
            return np.tile(w, (8, 1)).copy()

        cores.append(
            dict(
                SRCIDX=wrap16(srcidx),
                DSTIDX=wrap16(dstidx),
                DSTMOD=_bf16(
                    dstmod.reshape(spc * S, P).T
                ),  # [128, spc*S] col=subtile
                EAT=_bf16(np.moveaxis(eat, 0, 1).reshape(EFP, spc * C)),
            )
        )

    x = np.asarray(x, dtype=np.float32)
    xp = np.zeros((npad, P), dtype=np.float32)
    xp[:n_nodes, :NF] = x
    # node-packed: [p, (n//128)*128 + f] = xp[n, f] ; n%128 = p
    xpack = _bf16(xp.reshape(stripes, P, P).transpose(1, 0, 2).reshape(P, npad))
    for c in range(NCORES):
        cores[c]["SLICEPK"] = np.ascontiguousarray(
            xpack[:, c * npc : (c + 1) * npc]
        )
        hs = xp[c * npc : (c + 1) * npc, :NF]  # [npc, 92]
        cores[c]["HSLICE"] = np.ascontiguousarray(
            hs.reshape(spc, P, NF).transpose(1, 0, 2).reshape(P, spc * NF)
        )
        m = np.zeros((npad,), dtype=np.float32)
        m[:n_nodes] = 1.0
        cores[c]["MASKB"] = np.ascontiguousarray(
            m[c * npc : (c + 1) * npc].reshape(spc, P).T
        )
    iotaf = np.tile(np.arange(P, dtype=np.float32), (P, 8))
    meta = dict(spc=spc, npad=npad, npc=npc, split=split, HLO=HLO, HHI=HHI, C=C, S=S)
    shared = dict(
        XPACK=xpack,
        IOTAF=_bf16(iotaf),
        IDENT=np.eye(P, dtype=np.float32),
    )
    return meta, shared, cores


def pack_weights(inputs, n_nodes):
    """Conv + fc weights into device layouts (shared across cores)."""
    out = {}
    for c in (1, 2, 3):
        wf = np.asarray(inputs[f"conv{c}_Wf"], dtype=np.float32)
        ws = np.asarray(inputs[f"conv{c}_Ws"], dtype=np.float32)
        bf = np.asarray(inputs[f"conv{c}_bf"], dtype=np.float32)
        bs = np.asarray(inputs[f"conv{c}_bs"], dtype=np.float32)
        # F-half negated: the kernel computes exp(-F) and exp(S) in one
        # Exp op over the packed [-F | S] psum.
        wd = np.zeros((P, 2 * NF), dtype=np.float32)
        wsrc = np.zeros((P, 2 * NF), dtype=np.float32)
        we = np.zeros((EFP, 2 * NF), dtype=np.float32)
        wd[:NF, :NF] = -wf[:NF]
        wd[:NF, NF:] = ws[:NF]
        wsrc[:NF, :NF] = -wf[NF : 2 * NF]
        wsrc[:NF, NF:] = ws[NF : 2 * NF]
        we[:EF, :NF] = -wf[2 * NF :]
        we[:EF, NF:] = ws[2 * NF :]
        we[EFP - 1, :NF] = -bf
        we[EFP - 1, NF:] = bs
        out[f"WD{c - 1}"] = _bf16(wd)
        out[f"WSRC{c - 1}"] = _bf16(wsrc)
        out[f"WE{c - 1}"] = _bf16(we)
    out["FC1W"] = np.asarray(inputs["fc1_W"], dtype=np.float32)
    out["FC1B"] = np.asarray(inputs["fc1_b"], dtype=np.float32).reshape(NF, 1)
    for j in range(2, 8):
        out[f"FC{j}W"] = np.asarray(inputs[f"fc{j}_W"], dtype=np.float32)
        out[f"FC{j}B"] = np.asarray(inputs[f"fc{j}_b"], dtype=np.float32).reshape(
            -1, 1
        )
    ga = np.zeros((P, 1), dtype=np.float32)
    gb = np.zeros((52, 1), dtype=np.float32)
    g = np.asarray(inputs["glob_attr"], dtype=np.float32).reshape(-1)
    ga[: min(128, g.size), 0] = g[:128]
    gb[: max(0, g.size - 128), 0] = g[128:180]
    out["GA"] = ga
    out["GB"] = gb
    return out


def build(nc, meta, gf_dim):
    import os as _os

    N_CONVS = int(_os.environ.get("CGCNN_CONVS", "3"))
    DO_AG = int(_os.environ.get("CGCNN_AG", "1"))
    DO_HEAD = int(_os.environ.get("CGCNN_HEAD", "1"))
    spc, npad, npc = meta["spc"], meta["npad"], meta["npc"]
    split, HLO, HHI, C, S = (
        meta["split"],
        meta["HLO"],
        meta["HHI"],
        meta["C"],
        meta["S"],
    )
    G = S // 2  # psum groups (2 subtiles each)
    MB = 4  # groups per DVE batch (8 subtiles)
    n_nodes = meta["n_nodes"]

    dram = {}

    def din(name, shape, dt):
        dram[name] = nc.dram_tensor(name, shape, dt, kind="ExternalInput")
        return dram[name]

    din("XPACK", [P, npad], BF16)
    din("SLICEPK", [P, npc], BF16)
    din("HSLICE", [P, spc * NF], F32)
    din("SRCIDX", [P, spc * C // 16], I16)
    din("DSTIDX", [P, spc * C // 16], I16)
    din("DSTMOD", [P, spc * S], BF16)
    din("EAT", [EFP, spc * C], BF16)
    din("MASKB", [P, spc], F32)
    din("IOTAF", [P, 8 * P], BF16)
    din("IDENT", [P, P], F32)
    for c in range(3):
        din(f"WD{c}", [P, 2 * NF], BF16)
        din(f"WSRC{c}", [P, 2 * NF], BF16)
        din(f"WE{c}", [EFP, 2 * NF], BF16)
    din("FC1W", [NF, NF], F32)
    din("FC1B", [NF, 1], F32)
    fcdims = [(272, 1024), (1024, 512), (512, 256), (256, 128), (128, 64), (64, 1)]
    for j, (di, do) in enumerate(fcdims, start=2):
        din(f"FC{j}W", [di, do], F32)
        din(f"FC{j}B", [do, 1], F32)
    din("GA", [P, 1], F32)
    din("GB", [52, 1], F32)
    out_t = nc.dram_tensor("OUT", [1, 1], F32, kind="ExternalOutput")
    DBG = int(_os.environ.get("CGCNN_DBG", "0"))
    dbg_t = (
        nc.dram_tensor("DBG", [P, spc * NF], F32, kind="ExternalOutput")
        if DBG
        else None
    )
    slice_dram = nc.dram_tensor("slice_dram", [P, npc], BF16)
    gather_dram = nc.dram_tensor(
        "gather_dram", [NCORES * P, npc], BF16, addr_space="Shared"
    )
    pool_in_dram = nc.dram_tensor("pool_in_dram", [NF, 1], F32)
    pool_out_dram = nc.dram_tensor("pool_out_dram", [NF, 1], F32, addr_space="Shared")

    # persistent SBUF
    table = nc.alloc_sbuf_tensor("table", [P, npad], BF16)
    sliceT = nc.alloc_sbuf_tensor("sliceT", [P, npc], BF16)
    hfp = nc.alloc_sbuf_tensor("hfp", [P, spc * NF], F32)
    dstmod = nc.alloc_sbuf_tensor("dstmod", [P, spc * S], BF16)
    iotaf = nc.alloc_sbuf_tensor("iotaf", [P, 8 * P], BF16)
    ident = nc.alloc_sbuf_tensor("ident", [P, P], F32)
    maskb = nc.alloc_sbuf_tensor("maskb", [P, spc], F32)
    wsb = {}
    for c in range(3):
        wsb[f"WD{c}"] = nc.alloc_sbuf_tensor(f"wd{c}", [P, 2 * NF], BF16)
        wsb[f"WSRC{c}"] = nc.alloc_sbuf_tensor(f"wsrc{c}", [P, 2 * NF], BF16)
        wsb[f"WE{c}"] = nc.alloc_sbuf_tensor(f"we{c}", [EFP, 2 * NF], BF16)
    fc1w = nc.alloc_sbuf_tensor("fc1w", [NF, NF], F32)
    fc1b = nc.alloc_sbuf_tensor("fc1b", [NF, 1], F32)
    ga_sb = nc.alloc_sbuf_tensor("ga_sb", [P, 1], F32)
    gb_sb = nc.alloc_sbuf_tensor("gb_sb", [52, 1], F32)
    poolp = nc.alloc_sbuf_tensor("poolp", [P, 1], F32)
    poolf = nc.alloc_sbuf_tensor("poolf", [P, 1], F32)
    c20 = nc.alloc_sbuf_tensor("c20", [P, 1], F32)
    cm20 = nc.alloc_sbuf_tensor("cm20", [P, 1], F32)

    with tile.TileContext(nc) as tc:
        with (
            tc.tile_pool(name="gath", bufs=2) as gpool,
            tc.tile_pool(name="ea", bufs=2) as eapool,
            tc.tile_pool(name="stage", bufs=2) as stpool,
            tc.tile_pool(name="msgps", bufs=4, space="PSUM") as msgps,
            tc.tile_pool(name="aggps", bufs=2, space="PSUM") as aggps,
        ):
            # initial loads
            nc.sync.dma_start(out=table[:, :], in_=dram["XPACK"][:, :])
            nc.sync.dma_start(out=sliceT[:, :], in_=dram["SLICEPK"][:, :])
            nc.sync.dma_start(out=hfp[:, :], in_=dram["HSLICE"][:, :])
            nc.sync.dma_start(out=dstmod[:, :], in_=dram["DSTMOD"][:, :])
            nc.sync.dma_start(out=iotaf[:, :], in_=dram["IOTAF"][:, :])
            nc.sync.dma_start(out=ident[:, :], in_=dram["IDENT"][:, :])
            nc.sync.dma_start(out=maskb[:, :], in_=dram["MASKB"][:, :])
            for k, t in wsb.items():
                nc.sync.dma_start(out=t[:, :], in_=dram[k][:, :])
            nc.sync.dma_start(out=fc1w[:, :], in_=dram["FC1W"][:, :])
            nc.sync.dma_start(out=fc1b[:, :], in_=dram["FC1B"][:, :])
            nc.sync.dma_start(out=ga_sb[:, :], in_=dram["GA"][:, :])
            nc.sync.dma_start(out=gb_sb[:, :], in_=dram["GB"][:, :])

            nc.vector.memset(c20[:, :], 20.0)
            nc.vector.memset(cm20[:, :], -20.0)
            r_hlo = nc.gpsimd.to_reg(HLO)
            r_hhi = nc.gpsimd.to_reg(HHI)
            r_c = nc.gpsimd.to_reg(C)

            def emit_silu(out_ap, x_ap, pool, rows, cols, tag, bias_ap=None):
                # out = silu(x + b) = (x+b) * sigmoid(x+b), via Exp + recip.
                # Clamp to >= -80 so exp(-x) stays finite; silu(-80) ~ 0.
                xb = pool.tile([P, cols], F32, tag=f"sx{tag}")
                nc.vector.tensor_scalar(
                    out=xb[:rows, :cols],
                    in0=x_ap,
                    scalar1=bias_ap if bias_ap is not None else 0.0,
                    scalar2=-80.0,
                    op0=ALU.add,
                    op1=ALU.max,
                )
                x_ap = xb[:rows, :cols]
                e = pool.tile([P, cols], BF16, tag=f"se{tag}")
                nc.scalar.activation(
                    out=e[:rows, :cols], in_=x_ap, func=AF.Exp, scale=-1.0
                )
                with nc.allow_low_precision("bf16 silu"):
                    nc.vector.tensor_scalar_add(
                        e[:rows, :cols], e[:rows, :cols], 1.0
                    )
                    nc.vector.reciprocal(
                        e[:rows, :cols], e[:rows, :cols]
                    )
                nc.vector.tensor_tensor(
                    out=out_ap, in0=x_ap, in1=e[:rows, :cols], op=ALU.mult
                )

            for conv in range(N_CONVS):
                wd, wsrc, we = (
                    wsb[f"WD{conv}"],
                    wsb[f"WSRC{conv}"],
                    wsb[f"WE{conv}"],
                )
                for b in range(spc):
                    eat_t = eapool.tile([EFP, C], BF16, tag="eat")
                    nc.sync.dma_start(
                        out=eat_t[:, :], in_=dram["EAT"][:, b * C : (b + 1) * C]
                    )
                    i0 = b * C // 16
                    si_t = eapool.tile([P, C // 16], I16, tag="si")
                    di_t = eapool.tile([P, C // 16], I16, tag="di")
                    nc.sync.dma_start(
                        out=si_t[:, :], in_=dram["SRCIDX"][:, i0 : i0 + C // 16]
                    )
                    nc.sync.dma_start(
                        out=di_t[:, :], in_=dram["DSTIDX"][:, i0 : i0 + C // 16]
                    )
                    xdT = gpool.tile([P, C], BF16, tag="xd")
                    xsT = gpool.tile([P, C], BF16, tag="xs")
                    # dst gather (from own slice table)
                    nc.gpsimd.dma_gather(
                        xdT[:, :].rearrange("p (o n) -> p o n", o=1),
                        sliceT[:, :],
                        di_t[:, :],
                        C,
                        r_c,
                        P,
                        transpose=True,
                        single_packet=False,
                        sbuf_tokens_per_rank=P,
                        sbuf_free_dim_per_rank=2 * P,
                    )
                    # src gathers (lo/hi halves of the full table)
                    nc.gpsimd.dma_gather(
                        xsT[:, :HLO].rearrange("p (o n) -> p o n", o=1),
                        table[:, :split],
                        si_t[:, : HLO // 16],
                        HLO,
                        r_hlo,
                        P,
                        transpose=True,
                        single_packet=False,
                        sbuf_tokens_per_rank=P,
                        sbuf_free_dim_per_rank=2 * P,
                    )
                    nc.gpsimd.dma_gather(
                        xsT[:, HLO:].rearrange("p (o n) -> p o n", o=1),
                        table[:, split:],
                        si_t[:, HLO // 16 :],
                        HHI,
                        r_hhi,
                        P,
                        transpose=True,
                        single_packet=False,
                        sbuf_tokens_per_rank=P,
                        sbuf_free_dim_per_rank=2 * P,
                    )

                    agg = aggps.tile([P, NF], F32, tag="agg")
                    for gb0 in range(0, G, MB):
                        nb = min(MB, G - gb0)  # groups in this batch
                        ns = 2 * nb  # subtiles in this batch
                        sub0 = gb0 * 2
                        oh = stpool.tile([P, MB * 2, P], BF16, tag="oh")
                        eb = stpool.tile([P, MB * 2, 2 * NF], BF16, tag="eb")
                        lnb = stpool.tile([P, MB * 2, NF], BF16, tag="ln")
                        rsb = stpool.tile([P, MB * 2, NF], BF16, tag="rsb")
                        msg = stpool.tile([P, MB * 2, NF], BF16, tag="msg")
                        # one-hot for all subtiles in batch
                        mcol = b * S + sub0
                        nc.vector.tensor_tensor(
                            out=oh[:, :ns, :],
                            in0=dstmod[:, mcol : mcol + ns]
                            .rearrange("p (g o) -> p g o", o=1)
                            .broadcast_to([P, ns, P]),
                            in1=iotaf[:, : ns * P].rearrange(
                                "p (g n) -> p g n", n=P
                            ),
                            op=ALU.is_equal,
                        )
                        for q in range(nb):
                            g = gb0 + q
                            ps = msgps.tile([P, 2 * 2 * NF], F32, tag="msgps")
                            for t in range(2):
                                col = g * 256 + t * P
                                sl = slice(col, col + P)
                                po = ps[:, t * 2 * NF : (t + 1) * 2 * NF]
                                nc.tensor.matmul(
                                    out=po,
                                    lhsT=xdT[:, sl],
                                    rhs=wd[:, :],
                                    start=True,
                                    stop=False,
                                )
                                nc.tensor.matmul(
                                    out=po,
                                    lhsT=xsT[:, sl],
                                    rhs=wsrc[:, :],
                                    start=False,
                                    stop=False,
                                )
                                nc.tensor.matmul(
                                    out=po,
                                    lhsT=eat_t[:, sl],
                                    rhs=we[:, :],
                                    start=False,
                                    stop=True,
                                )
                            ps4 = ps[:, :].rearrange(
                                "p (t f) -> p t f", f=2 * NF
                            )
                            qs = slice(2 * q, 2 * q + 2)
                            # psum = [-F | S].
                            # sigmoid(F) = 1/(1+exp(-F)); exp overflow to inf
                            # is fine (recip -> 0).
                            nc.scalar.activation(
                                out=eb[:, qs, :NF],
                                in_=ps4[:, :, :NF],
                                func=AF.Exp,
                            )
                            # stable softplus(S) =
                            #   softplus(min(S,20)) + relu(S-20):
                            #   r2 = relu(20-S); u0 = exp(-r2)
                            #   softplus(min(S,20)) = ln(1 + e^20 * u0)
                            r2 = stpool.tile([P, 2, NF], BF16, tag="r2")
                            nc.scalar.activation(
                                out=r2[:, :, :],
                                in_=ps4[:, :, NF:],
                                func=AF.Relu,
                                scale=-1.0,
                                bias=c20[:, :],
                            )
                            nc.scalar.activation(
                                out=eb[:, qs, NF:],
                                in_=r2[:, :, :],
                                func=AF.Exp,
                                scale=-1.0,
                            )
                            nc.scalar.activation(
                                out=lnb[:, qs, :],
                                in_=eb[:, qs, NF:],
                                func=AF.Ln,
                                scale=float(np.exp(20.0)),
                                bias=1.0,
                            )
                            nc.scalar.activation(
                                out=rsb[:, qs, :],
                                in_=ps4[:, :, NF:],
                                func=AF.Relu,
                                bias=cm20[:, :],
                            )
                        # sigmoid(F) = 1/(1+t); msg = sigmoid * softplus
                        with nc.allow_low_precision("bf16 gate math"):
                            nc.vector.tensor_scalar_add(
                                eb[:, :ns, :NF], eb[:, :ns, :NF], 1.0
                            )
                            nc.vector.reciprocal(
                                eb[:, :ns, :NF], eb[:, :ns, :NF]
                            )
                            nc.vector.tensor_tensor(
                                out=lnb[:, :ns, :],
                                in0=lnb[:, :ns, :],
                                in1=rsb[:, :ns, :],
                                op=ALU.add,
                            )
                        nc.vector.tensor_tensor(
                            out=msg[:, :ns, :],
                            in0=eb[:, :ns, :NF],
                            in1=lnb[:, :ns, :],
                            op=ALU.mult,
                        )
                        for q in range(ns):
                            nc.tensor.matmul(
                                out=agg[:, :],
                                lhsT=oh[:, q, :],
                                rhs=msg[:, q, :],
                                start=(sub0 + q == 0),
                                stop=(sub0 + q == S - 1),
                            )
                    # flush block b
                    ftmp = stpool.tile([P, NF], F32, tag="ftmp")
                    hsl = hfp[:, b * NF : (b + 1) * NF]
                    nc.vector.tensor_tensor(
                        out=ftmp[:, :], in0=agg[:, :], in1=hsl, op=ALU.add
                    )
                    emit_silu(hsl, ftmp[:, :], stpool, P, NF, "fl")
                    if conv < 2:
                        nc.vector.tensor_copy(
                            out=sliceT[:, b * P : b * P + NF], in_=hsl
                        )
                if conv < 2 and DO_AG:
                    nc.sync.dma_start(out=slice_dram[:, :], in_=sliceT[:, :])
                    nc.gpsimd.collective_compute(
                        "AllGather",
                        ALU.bypass,
                        replica_groups=[list(range(NCORES))],
                        ins=[slice_dram[:, :]],
                        outs=[gather_dram[:, :]],
                    )
                    nc.sync.dma_start(
                        out=table[:, :].rearrange("p (c j) -> p c j", c=NCORES),
                        in_=gather_dram[:, :].rearrange(
                            "(c p) j -> p c j", p=P
                        ),
                    )

        # ---- head ----
        if dbg_t is not None:
            nc.sync.dma_start(out=dbg_t[:, :], in_=hfp[:, :])
        if not DO_HEAD:
            with tc.tile_pool(name="dummy", bufs=1) as dpool:
                dt_ = dpool.tile([1, 1], F32, tag="d")
                nc.vector.tensor_copy(out=dt_[:1, :1], in_=hfp[:1, :1])
                nc.sync.dma_start(out=out_t[:, :], in_=dt_[:1, :1])
            return dram
        with (
            tc.tile_pool(name="mst", bufs=2) as mstpool,
            tc.tile_pool(name="wst", bufs=3) as wstpool,
            tc.tile_pool(name="hps", bufs=2, space="PSUM") as hps,
        ):
            # conv phase is done with the bf16 table; reuse its SBUF as the
            # fp32 feature-major h3 buffer for the head.
            h3T = table[:, :].bitcast(F32)[:, : spc * P]
            for b in range(spc):
                mtmp = mstpool.tile([P, NF], F32, tag="mtmp")
                nc.vector.tensor_tensor(
                    out=mtmp[:, :],
                    in0=hfp[:, b * NF : (b + 1) * NF],
                    in1=maskb[:, b : b + 1].broadcast_to([P, NF]),
                    op=ALU.mult,
                )
                tps = hps.tile([P, P], F32, tag="tps")
                nc.tensor.transpose(
                    out=tps[:NF, :], in_=mtmp[:, :], identity=ident[:, :]
                )
                nc.vector.tensor_copy(
                    out=h3T[:NF, b * P : (b + 1) * P], in_=tps[:NF, :]
                )
            # fc1 twice, in place over h3T chunks
            for _r in range(2):
                for ck in range(0, spc * P, 512):
                    n = min(512, spc * P - ck)
                    p1 = hps.tile([NF, 512], F32, tag="p1")
                    nc.tensor.matmul(
                        out=p1[:, :n],
                        lhsT=fc1w[:, :],
                        rhs=h3T[:NF, ck : ck + n],
                        start=True,
                        stop=True,
                    )
                    emit_silu(
                        h3T[:NF, ck : ck + n],
                        p1[:, :n],
                        mstpool,
                        NF,
                        n,
                        "fc1",
                        bias_ap=fc1b[:, :],
                    )
            nc.vector.tensor_reduce(
                out=poolp[:NF, :],
                in_=h3T[:NF, :],
                axis=mybir.AxisListType.X,
                op=ALU.add,
            )
            nc.scalar.mul(poolp[:NF, :], poolp[:NF, :], 1.0 / n_nodes)
            nc.sync.dma_start(out=pool_in_dram[:, :], in_=poolp[:NF, :])
            nc.gpsimd.collective_compute(
                "AllReduce",
                ALU.add,
                replica_groups=[list(range(NCORES))],
                ins=[pool_in_dram[:, :]],
                outs=[pool_out_dram[:, :]],
            )
            nc.sync.dma_start(out=poolf[:NF, :], in_=pool_out_dram[:, :])
            # MLP chain
            fcdims2 = [
                (272, 1024),
                (1024, 512),
                (512, 256),
                (256, 128),
                (128, 64),
                (64, 1),
            ]
            gcur = None  # sbuf tile [128, mtiles] fp32, col m = features m*128..
            for j, (di, do) in enumerate(fcdims2, start=2):
                mt = (do + P - 1) // P
                gnext = mstpool.tile([P, max(mt, 1)], F32, tag=f"g{j}")
                if j == 2:
                    ktiles = [
                        (0, NF, poolf[:NF, :]),
                        (NF, NF + P, ga_sb[:, :]),
                        (NF + P, 272, gb_sb[:, :]),
                    ]
                else:
                    ktiles = [
                        (k * P, min((k + 1) * P, di), gcur[:, k : k + 1])
                        for k in range((di + P - 1) // P)
                    ]
                for m in range(mt):
                    m0, m1 = m * P, min((m + 1) * P, do)
                    pm = hps.tile([P, 1], F32, tag="pm")
                    for ki, (k0, k1, rhs) in enumerate(ktiles):
                        wst = wstpool.tile([P, P], F32, tag="wst")
                        nc.sync.dma_start(
                            out=wst[: k1 - k0, : m1 - m0],
                            in_=dram[f"FC{j}W"][k0:k1, m0:m1],
                        )
                        nc.tensor.matmul(
                            out=pm[: m1 - m0, :],
                            lhsT=wst[: k1 - k0, : m1 - m0],
                            rhs=rhs[: k1 - k0, :],
                            start=(ki == 0),
                            stop=(ki == len(ktiles) - 1),
                        )
                    bst = wstpool.tile([P, 1], F32, tag="bst")
                    nc.sync.dma_start(
                        out=bst[: m1 - m0, :], in_=dram[f"FC{j}B"][m0:m1, :]
                    )
                    if j < 7:
                        emit_silu(
                            gnext[: m1 - m0, m : m + 1],
                            pm[: m1 - m0, :],
                            mstpool,
                            m1 - m0,
                            1,
                            "mlp",
                            bias_ap=bst[: m1 - m0, :],
                        )
                    else:
                        nc.vector.tensor_tensor(
                            out=gnext[: m1 - m0, m : m + 1],
                            in0=pm[: m1 - m0, :],
                            in1=bst[: m1 - m0, :],
                            op=ALU.add,
                        )
                gcur = gnext
            nc.sync.dma_start(out=out_t[:, :], in_=gcur[:1, :1])

    return dram


LAST_EXEC_NS = None


def prepare(**inputs):
    x = np.asarray(inputs["x"])
    edge_index = np.asarray(inputs["edge_index"])
    edge_attr = np.asarray(inputs["edge_attr"])
    n_nodes, n_edges = x.shape[0], edge_index.shape[1]

    meta, shared, cores = preprocess(x, edge_index, edge_attr, n_nodes, n_edges)
    meta["n_nodes"] = n_nodes
    w = pack_weights(inputs, n_nodes)

    nc = bacc.Bacc(
        "TRN2",
        target_bir_lowering=False,
        debug=False,
        enable_asserts=False,
        num_devices=NCORES,
    )
    build(nc, meta, None)
    nc.compile()

    in_maps = []
    for c in range(NCORES):
        m = dict(shared)
        m.update(w)
        m.update(cores[c])
        m = {k: np.asarray(v) for k, v in m.items()}
        in_maps.append(m)
    return nc, in_maps


def run(nc, in_maps):
    global LAST_EXEC_NS
    import os

    trace = bool(int(os.environ.get("CGCNN_TRACE", "0")))
    res = run_bass_kernel_spmd(
        nc, in_maps, core_ids=list(range(NCORES)), trace=trace
    )
    LAST_EXEC_NS = res.exec_time_ns
    out = res.results[0]["OUT"]
    return np.asarray(out, dtype=np.float32).reshape(1, 1)


def kernel(**inputs):
    nc, in_maps = prepare(**inputs)
    return run(nc, in_maps)


if __name__ == "__main__":
    # tiny smoke: random small graph vs numpy reference
    pass
